# revision 20
# baseline (speedup 1.0000x reference)
"""NT-Xent loss kernel for Trainium2, 8 NeuronCores.

Problem: B=4096 per view, D=128, temperature=0.1.
reps = concat([zjs, zis]) -> [8192, 128]; normalize rows; sim = normed @ normed.T;
loss = mean_i(-pos_i/T + logsumexp_{j!=i}(sim_ij/T)).

Strategy (fully static SPMD, no collectives):
  Each core c receives reps rotated by -1024*c rows, host-packed into the
  on-chip tile layout [p, t, d] as bf16 (the matmul operand precision anyway)
  so the input load is 2MB of fully-contiguous DMA, split across the two
  HWDGE queues (SP + ACT engines).  In the rotated frame the core's 1024 rows
  are rows 0..1023, the diagonal for row-chunk mi sits at columns
  128mi..128mi+127 of column-group 0, and the positive column for row
  128mi+p is 4096+128mi+p.  Per core:
    Phase A: row sums of squares (fp32 accumulation of exact bf16 products),
             rsqrt via exp(-0.5*ln(ss)) (single ACT table set), normalize to
             bf16 via one broadcast-AP multiply per 4-tile slice, xbar-DMA-
             transpose slices into HIT [128d, 8192rows] (bf16).
             diag_i = ||hi_i||^2 and pos_i come from row-major dots emitted
             early so they never gate the ScalarE pipeline.
    Phase B: per column group q (4) x row-chunk mi (8): 4 bf16 matmuls
             (N=512) -> PSUM [128, 2048]; ScalarE Exp(10x-10) in place with
             accum_out producing the partial row sums of exp.
  Per-row bf16 noise (~2e-3) is zero-mean; averaged over 8192 rows the final
  scalar keeps ~1e-5 relative accuracy.  Output per core: [128, 8] per-row
  losses; host sums in float64.
"""

import numpy as np

B = 4096
D = 128
TWO_B = 2 * B
P = 128
NTILE = TWO_B // P        # 64 row tiles
MI = 8                    # row-chunks per core (128 rows each -> 1024 rows)
GQ = 4                    # column groups of 2048
GW = TWO_B // GQ          # 2048 columns per group
TPG = GW // P             # 16 tiles per group
SUB = 4                   # tiles per fine-grained norm/transpose slice
NCORES = 8
ROWS_PER_CORE = TWO_B // NCORES  # 1024
INV_T = 10.0              # 1 / temperature
SHIFT = 10.0              # fixed logsumexp shift (sim/T <= 10)

_CACHE = {}


def build_nc():
    import concourse.bacc as bacc
    import concourse.bass as bass
    import concourse.mybir as mybir
    import concourse.tile as tile

    f32 = mybir.dt.float32
    bf16 = mybir.dt.bfloat16
    AX = mybir.AxisListType
    OP = mybir.AluOpType
    AF = mybir.ActivationFunctionType

    # Make the act-table chooser pick the one set that holds BOTH Ln and Exp
    # (otherwise it alternates exp_and_others <-> natural_log, ~1.3us per
    # reload).  Only the choice is restricted; the chosen set's real runtime
    # contents still cover every function we emit.
    from concourse import hw_specs

    _orig_tables = hw_specs.get_activation_tables

    def _patched_tables(arch):
        t = {k: set(v) for k, v in _orig_tables(arch).items()}
        for name, s in t.items():
            if name != "natural_log_exp_and_others":
                s.discard(AF.Exp)
                s.discard(AF.Ln)
        return t

    bacc.get_activation_tables = _patched_tables

    nc = bacc.Bacc(
        "TRN2",
        target_bir_lowering=False,
        debug=False,
        num_devices=NCORES,
    )
    # host-tiled: reps_h[p, t*128 + d] = bf16(reps_rot[t*128 + p, d])
    reps_h = nc.declare_dram_parameter("reps", [P, TWO_B], bf16, isOutput=False)
    out_h = nc.declare_dram_parameter("out", [P, MI], f32, isOutput=True)

    ident_dram = nc.inline_tensor(np.eye(P, dtype=np.float32), name="ident_const")

    with tile.TileContext(nc) as tc:
        with (
            tc.tile_pool(name="persist", bufs=1) as persist,
            tc.tile_pool(name="psum", bufs=2, space="PSUM") as psum,
            tc.tile_pool(name="scratch", bufs=2) as scratch,
        ):
            ident = persist.tile([P, P], f32)
            nc.gpsimd.dma_start(out=ident, in_=ident_dram[:, :])
            bias_shift = persist.tile([P, 1], f32)
            nc.vector.memset(bias_shift, -SHIFT)

            RAW = persist.tile([P, NTILE, P], bf16)
            SQ = persist.tile([P, NTILE, P], f32)
            HI = persist.tile([P, NTILE, P], bf16)
            HIT = persist.tile([P, TWO_B], bf16)
            SS = persist.tile([P, NTILE], f32)
            SSC = persist.tile([P, NTILE], f32)
            LNSS = persist.tile([P, NTILE], f32)
            SCL = persist.tile([P, NTILE], f32)
            SPART = persist.tile([P, MI, GQ], f32)
            DIAG = persist.tile([P, MI], f32)
            POS = persist.tile([P, MI], f32)

            # ---------------- Phase A: load + normalize + transpose ---------
            reps_t = reps_h[:, :].rearrange("p (t d) -> p t d", d=P)
            HIT3 = HIT.rearrange("d (t p) -> d t p", p=P)

            def scl_bcast(a, b):
                # SCL[:, a:b] broadcast along a trailing step-0 dim of width P
                s = SCL[:, a:b]
                return bass.AP(
                    tensor=s.tensor, offset=s.offset, ap=list(s.ap) + [[0, P]]
                )

            def slice_stats(x, y):
                nc.vector.tensor_mul(SQ[:, x:y, :], RAW[:, x:y, :], RAW[:, x:y, :])
                nc.vector.reduce_sum(out=SS[:, x:y], in_=SQ[:, x:y, :], axis=AX.X)
                # norm clamp: max(||r||, 1e-8) == sqrt(max(ss, 1e-16))
                nc.vector.tensor_scalar_max(out=SSC[:, x:y], in0=SS[:, x:y],
                                            scalar1=1e-16)
                # rsqrt(ss) = exp(-0.5*ln(ss))
                nc.scalar.activation(out=LNSS[:, x:y], in_=SSC[:, x:y], func=AF.Ln)
                nc.scalar.activation(out=SCL[:, x:y], in_=LNSS[:, x:y],
                                     func=AF.Exp, scale=-0.5)

            def slice_norm(x, y, eng):
                nc.vector.tensor_mul(HI[:, x:y, :], RAW[:, x:y, :], scl_bcast(x, y))
                eng.dma_start_transpose(out=HIT3[:, x:y, :], in_=HI[:, x:y, :])

            def phase_a_group0():
                # group 0 gates Phase B: 4-tile pieces on alternating load
                # queues, per-piece stats, transposes split over BOTH HWDGE
                # engines (ScalarE is idle this early)
                for s, (ld, tr) in enumerate(
                    [(nc.scalar, nc.sync), (nc.gpsimd, nc.scalar)] * 2
                ):
                    x, y = s * SUB, (s + 1) * SUB
                    ld.dma_start(out=RAW[:, x:y, :], in_=reps_t[:, x:y, :])
                    slice_stats(x, y)
                    slice_norm(x, y, tr)

            def phase_a_group(g):
                a, b = g * TPG, (g + 1) * TPG
                # loads only on the ACT-HWDGE and SWDGE queues: the sync
                # queue is reserved for the transposes so group 0's
                # transposes aren't stuck behind groups 1-3's loads
                splits = [(a, a + 10, nc.scalar), (a + 10, b, nc.gpsimd)]
                for x, y, eng in splits:
                    eng.dma_start(out=RAW[:, x:y, :], in_=reps_t[:, x:y, :])
                for x, y, _ in splits:
                    nc.vector.tensor_mul(
                        SQ[:, x:y, :], RAW[:, x:y, :], RAW[:, x:y, :]
                    )
                    nc.vector.reduce_sum(
                        out=SS[:, x:y], in_=SQ[:, x:y, :], axis=AX.X
                    )
                nc.vector.tensor_scalar_max(
                    out=SSC[:, a:b], in0=SS[:, a:b], scalar1=1e-16
                )
                nc.scalar.activation(out=LNSS[:, a:b], in_=SSC[:, a:b], func=AF.Ln)
                nc.scalar.activation(
                    out=SCL[:, a:b], in_=LNSS[:, a:b], func=AF.Exp, scale=-0.5
                )
                for x in range(a, b, SUB):
                    nc.vector.tensor_mul(
                        HI[:, x : x + SUB, :],
                        RAW[:, x : x + SUB, :],
                        scl_bcast(x, x + SUB),
                    )
                    nc.sync.dma_start_transpose(
                        out=HIT3[:, x : x + SUB, :], in_=HI[:, x : x + SUB, :]
                    )

            phase_a_group0()
            # diag_i = ||hi_i||^2 exactly as the matmul computes it (same bf16
            # inputs, fp32 accumulation).  Emitted right after group 0 so the
            # ScalarE Phase B pipeline is never gated on late DVE work.
            for mi in range(MI):
                jd = scratch.tile([P, P], f32, tag="ttr_junk")
                nc.vector.scalar_tensor_tensor(
                    out=jd,
                    in0=HI[:, mi, :],
                    scalar=1.0,
                    in1=HI[:, mi, :],
                    op0=OP.mult,
                    op1=OP.mult,
                    accum_out=DIAG[:, mi : mi + 1],
                )
            phase_a_group(1)
            phase_a_group(2)
            # positive-pair dots: row-chunk mi pairs tile mi with tile 32+mi
            for mi in range(MI):
                jp = scratch.tile([P, P], f32, tag="ttr_junk")
                nc.vector.scalar_tensor_tensor(
                    out=jp,
                    in0=HI[:, mi, :],
                    scalar=1.0,
                    in1=HI[:, NTILE // 2 + mi, :],
                    op0=OP.mult,
                    op1=OP.mult,
                    accum_out=POS[:, mi : mi + 1],
                )
            phase_a_group(3)

            # ---------------- Phase B: sim row-blocks + exp row-sums --------
            for q in range(GQ):
                for mi in range(MI):
                    lhsT = HIT[:, mi * P : (mi + 1) * P]
                    pg = psum.tile([P, GW], f32, tag="pg")
                    for k in range(GW // 512):
                        nc.tensor.matmul(
                            pg[:, k * 512 : (k + 1) * 512],
                            lhsT,
                            HIT[:, q * GW + k * 512 : q * GW + (k + 1) * 512],
                            start=True,
                            stop=True,
                        )
                    # exp(10*sim - 10) in place on PSUM; accum_out = row sum
                    nc.scalar.activation(
                        out=pg,
                        in_=pg,
                        func=AF.Exp,
                        scale=INV_T,
                        bias=bias_shift,
                        accum_out=SPART[:, mi, q : q + 1],
                    )

            # ---------------- tail: per-row loss -----------------------------
            STOT = persist.tile([P, MI], f32)
            DEXP = persist.tile([P, MI], f32)
            SSUB = persist.tile([P, MI], f32)
            LNS = persist.tile([P, MI], f32)
            OUTA = persist.tile([P, MI], f32)
            OUTF = persist.tile([P, MI], f32)

            nc.vector.reduce_sum(out=STOT, in_=SPART, axis=AX.X)
            nc.scalar.activation(
                out=DEXP, in_=DIAG, func=AF.Exp, scale=INV_T, bias=bias_shift
            )
            nc.vector.tensor_sub(SSUB, STOT, DEXP)
            nc.scalar.activation(out=LNS, in_=SSUB, func=AF.Ln)
            # loss = ln(sum) + SHIFT - INV_T * pos
            nc.vector.scalar_tensor_tensor(
                out=OUTA,
                in0=POS,
                scalar=-INV_T,
                in1=LNS,
                op0=OP.mult,
                op1=OP.add,
            )
            nc.vector.tensor_scalar_add(out=OUTF, in0=OUTA, scalar1=SHIFT)
            nc.sync.dma_start(out=out_h[:, :], in_=OUTF)

    nc.compile()
    return nc


def get_nc():
    if "nc" not in _CACHE:
        _CACHE["nc"] = build_nc()
    return _CACHE["nc"]


def make_in_maps(zis: np.ndarray, zjs: np.ndarray):
    import ml_dtypes

    # representations in reference order: [zjs; zis]
    reps = np.concatenate(
        [np.asarray(zjs, np.float32), np.asarray(zis, np.float32)], axis=0
    )
    maps = []
    for c in range(NCORES):
        rot = np.roll(reps, -ROWS_PER_CORE * c, axis=0)
        tiled = np.ascontiguousarray(
            rot.reshape(NTILE, P, D).transpose(1, 0, 2).reshape(P, TWO_B)
        ).astype(ml_dtypes.bfloat16)
        maps.append({"reps": tiled})
    return maps


def kernel(zis: np.ndarray, zjs: np.ndarray) -> np.ndarray:
    from concourse.bass_utils import run_bass_kernel_spmd

    nc = get_nc()
    in_maps = make_in_maps(zis, zjs)
    res = None
    for attempt in range(3):
        try:
            res = run_bass_kernel_spmd(nc, in_maps, core_ids=list(range(NCORES)))
            break
        except Exception:
            # transient device-unrecoverable states heal on re-execution
            if attempt == 2:
                raise
            import time as _time

            _time.sleep(5.0)
    total = 0.0
    for r in res.results:
        total += float(r["out"].astype(np.float64).sum())
    return np.array(total / TWO_B, dtype=np.float32)
